# revision 9
# baseline (speedup 1.0000x reference)
"""Trainium2 Bass kernel for nn_AlignModel.

Computes out[b, j, i] = sigmoid(simp[b,j]·w_s + orig[b,i]·w_o + bias) where
orig/simp are the two halves of prop_state[b] ([B, 2S, D] -> [B,S,D] each),
w_o = W[0,:D], w_s = W[0,D:].

Sharding: data-parallel over batch B=8 across the 8 NeuronCores.  Host-side
staging per core (layout only -- all compute is on device):
  xot  [512, 2048] f16 = orig(b).T           (d-major, so PE can contract d)
  xs   [2048, 512] f16 = simp(b), rows permuted so HBM row p*16+n holds
        simp row n*128+p (partition-inner layout, contiguous descriptor lines)
  wsr  [128, 512] f16 = w_s replicated on all partitions
  wot  [128, 4]  f32 = w_o chunk-major (wot[k,e] = w_o[e*128+k])
  bvec [1, 1]   f32

Two-lane tile production (the baseline's single-ScalarE sigmoid chain was the
32us critical path; here ScalarE+GpSimd produce half the tiles and DVE the
other half, all fed from one PSUM s_o broadcast):
  - PE: psum_so[p,i] = s_o[i] via 16 fp16 matmuls with wrep stationary
    (wrep built on-device from wot by DVE; replication broadcasts s_o to
    all 128 partitions).
  - DVE dots: s_sb_mat[p,t] = simp[t*128+p]·w_s + b via one fused
    tensor_tensor_reduce per row-tile (b seeded as the reduce init).
  - Lane A (tiles 0..NS-1): ScalarE tanh(0.5*s_o + 0.5*(ss+b)) -> f32
    scratch (f32 avoids the fp16 (1+T)/2 cancellation; scores reach -4.4
    where sigma~0.012), then GpSimd affine 0.5*T+0.5 -> fp16 out tile.
    tanh and exp share one ACT table set => zero table switches.
  - Lane B (tiles NS..15): out = 1/(1 + exp(-ss-b)*exp(-s_o)); ScalarE
    makes eo/es once (2.3us), DVE does a fused (eo*es_col)+1 tensor_scalar
    (4x fp16 rate) + reciprocal per tile.  E <= exp(4.43) ~ 84, fp16-safe.
  - Stores: lane A rides the scalar HWDGE queue interleaved lag-1 with the
    tanh ACTIVATEs (each dma's producer finished >=1 op earlier => no ACT
    sequencer stalls); lane B rides the sync queue after the loads, in DVE
    completion order.  Per-store 0.5-1 MiB keeps descriptors efficient.
"""

import numpy as np

import concourse.mybir as mybir
from concourse import bacc, bass_utils
from concourse.tile import TileContext

P = 128          # partitions
D = 512          # feature dim
S = 2048         # sents
NT = S // P      # 16 row-tiles
NE = D // P      # 4 contraction chunks
NS = 9           # lane-A (ScalarE tanh) tiles; lane B (DVE) gets NT-NS
NCORES = 8
F32 = mybir.dt.float32
F16 = mybir.dt.float16
AF = mybir.ActivationFunctionType
ALU = mybir.AluOpType


def _kernel_body(tc, out, xot, xs, wsr, wot, bvec):
    nc = tc.nc
    xs_re = xs.rearrange("(p n) d -> p n d", n=NT)

    with (
        tc.tile_pool(name="consts", bufs=1) as cpool,
        tc.tile_pool(name="xin", bufs=1) as xpool,
        tc.tile_pool(name="prod", bufs=2) as prpool,
        tc.tile_pool(name="tpool", bufs=3) as tpool,
        tc.tile_pool(name="epool", bufs=2) as epool,
        tc.tile_pool(name="outbuf", bufs=1) as opool,
        tc.tile_pool(name="psum", bufs=1, space="PSUM") as ppool,
    ):
        # preload the exp_and_others ACT table set (covers Exp AND Tanh;
        # the whole kernel stays in this set -> no mid-kernel table loads).
        dummy = cpool.tile([1, 1], F32, tag="dummy")
        nc.vector.memset(dummy, 0.0)
        nc.scalar.activation(dummy, dummy, AF.Exp)

        # tiny const loads on the scalar HWDGE queue (lands ~9us, before
        # the first ttr dot / matmul needs them)
        b_sb = cpool.tile([P, 1], F32, tag="bsb")
        wsr_sb = cpool.tile([P, D], F16, tag="wsr")
        wot_sb = cpool.tile([P, NE], F32, tag="wot")
        nc.scalar.dma_start(out=wsr_sb, in_=wsr)
        nc.scalar.dma_start(out=wot_sb, in_=wot)
        nc.scalar.dma_start(out=b_sb, in_=bvec.broadcast_to([P, 1]))

        # build wrep on device: wrep[k, e*128+m] = w_o[e*128+k] (stationary
        # replicated along the PE output dim m so the matmul broadcasts s_o
        # to all partitions).  Replaces the baseline's 256KB wcat HBM load.
        ones = cpool.tile([P, P], F16, tag="ones")
        wrep_sb = cpool.tile([P, NE, P], F16, tag="wrep")
        nc.vector.memset(ones, 1.0)
        for e in range(NE):
            nc.vector.tensor_scalar_mul(wrep_sb[:, e, :], ones,
                                        wot_sb[:, e:e + 1])

        # --- input stream (sync queue, FIFO): xot e0,e1, xs[0:2] (un-gates
        # the first dots), xot e2,e3, xs rest in groups ---
        xs_all = xpool.tile([P, NT, D], F16, tag="xs")
        xot_all = xpool.tile([P, NE, S], F16, tag="xot")
        XS_GROUPS = [(0, 2), (2, 6), (6, 10), (10, NT)]
        for e in range(NE):
            nc.sync.dma_start(out=xot_all[:, e, :],
                              in_=xot[e * P:(e + 1) * P, :])
            if e == 1:
                nc.sync.dma_start(out=xs_all[:, 0:2, :], in_=xs_re[:, 0:2, :])
        for lo, hi in XS_GROUPS[1:]:
            nc.sync.dma_start(out=xs_all[:, lo:hi, :], in_=xs_re[:, lo:hi, :])

        s_sb_mat = cpool.tile([P, NT], F32, tag="ssmat")    # ss + b
        ss_half = cpool.tile([P, NT], F32, tag="sshalf")    # 0.5*(ss+b)
        es_mat = cpool.tile([P, NT - NS], F32, tag="esmat")  # exp(-(ss+b))
        eo_sb = cpool.tile([P, S], F16, tag="eo")            # exp(-s_o)
        so_psum = ppool.tile([P, S], F32, tag="so")

        # --- PE: s_o broadcast into PSUM ---
        for e in range(NE):
            for j in range(S // 512):
                nc.tensor.matmul(so_psum[:, j * 512:(j + 1) * 512],
                                 wrep_sb[:, e, :],
                                 xot_all[:, e, j * 512:(j + 1) * 512],
                                 start=(e == 0), stop=(e == NE - 1))

        # --- DVE dots: fp16 mul (2x rate) + grouped f32 reduce per load
        # group, then fold b and prescale 0.5 for the tanh bias.
        # (tensor_tensor_reduce would fuse these but hangs TRN2 HW.)
        for gi, (lo, hi) in enumerate(XS_GROUPS):
            g = hi - lo
            prod = prpool.tile([P, 6, D], F16, tag="prod", name=f"pr{gi}")
            for blk in range(g):
                nc.vector.tensor_mul(out=prod[:, blk, :],
                                     in0=xs_all[:, lo + blk, :], in1=wsr_sb)
            nc.vector.tensor_reduce(
                s_sb_mat[:, lo:hi], prod[:, 0:g, :],
                axis=mybir.AxisListType.X, op=ALU.add)
            nc.vector.tensor_scalar_add(s_sb_mat[:, lo:hi],
                                        s_sb_mat[:, lo:hi], b_sb)
            nc.vector.tensor_scalar_mul(ss_half[:, lo:hi],
                                        s_sb_mat[:, lo:hi], 0.5)

        out_all = opool.tile([P, NT, S], F16, tag="oall")

        # --- ScalarE program (one queue, issue order is execution order):
        # T0, T1, dmaA0, eo, dmaA1, es, T2, T3, dmaA2, T4, dmaA3, ...
        # lag-1 store interleave: dmaA_k's producer (GpSimd affine_k)
        # finished while the next tanh ran, so the ACT sequencer never
        # stalls on a dma_start's semaphore.
        def lane_a_tile(t):
            T = tpool.tile([P, S], F32, tag="T", name=f"T{t}")
            nc.scalar.activation(T, so_psum, AF.Tanh,
                                 bias=ss_half[:, t:t + 1], scale=0.5)
            # GpSimd affine: out tile = 0.5*T + 0.5 (f32 in -> f16 out)
            nc.gpsimd.tensor_scalar(out=out_all[:, t, :], in0=T,
                                    scalar1=0.5, scalar2=0.5,
                                    op0=ALU.mult, op1=ALU.add)

        def store_a(t):
            nc.scalar.dma_start(out=out[t * P:(t + 1) * P, :],
                                in_=out_all[:, t, :])

        lane_a_tile(0)
        lane_a_tile(1)
        store_a(0)
        nc.scalar.activation(eo_sb, so_psum, AF.Exp, scale=-1.0)
        store_a(1)
        nc.scalar.activation(es_mat, s_sb_mat[:, NS:NT], AF.Exp, scale=-1.0)
        lane_a_tile(2)
        for t in range(3, NS):
            lane_a_tile(t)
            store_a(t - 1)
        store_a(NS - 1)

        # --- DVE lane B: fused (eo*es_col)+1 at 4x fp16 rate, then
        # reciprocal (fp16 out; values in (0,1], |err| ~ 5e-4).
        with nc.allow_low_precision(reason="sigmoid output tile in fp16"):
            for t in range(NS, NT):
                E = epool.tile([P, S], F16, tag="E", name=f"E{t}")
                nc.vector.tensor_scalar(out=E, in0=eo_sb,
                                        scalar1=es_mat[:, t - NS:t - NS + 1],
                                        scalar2=1.0,
                                        op0=ALU.mult, op1=ALU.add)
                nc.vector.reciprocal(out_all[:, t, :], E)
                k = t - NS
                if k % 2 == 1 or t == NT - 1:
                    t0 = t - 1 if k % 2 == 1 else t
                    gsz = 2 if k % 2 == 1 else 1
                    r0 = t0 * P
                    if gsz == 1:
                        nc.sync.dma_start(out=out[r0:r0 + P, :],
                                          in_=out_all[:, t0, :])
                    else:
                        dst = out[r0:r0 + gsz * P, :].rearrange(
                            "(q p) i -> p q i", p=P)
                        nc.sync.dma_start(
                            out=dst, in_=out_all[:, t0:t0 + gsz, :])


def build_program():
    nc = bacc.Bacc(
        "TRN2",
        debug=False,
        target_bir_lowering=False,
        num_devices=NCORES,
    )
    xot = nc.dram_tensor("xot", [D, S], F16, kind="ExternalInput").ap()
    xs = nc.dram_tensor("xs", [S, D], F16, kind="ExternalInput").ap()
    wsr = nc.dram_tensor("wsr", [P, D], F16, kind="ExternalInput").ap()
    wot = nc.dram_tensor("wot", [P, NE], F32, kind="ExternalInput").ap()
    bvec = nc.dram_tensor("bvec", [1, 1], F32, kind="ExternalInput").ap()
    out = nc.dram_tensor("out", [S, S], F16, kind="ExternalOutput").ap()
    with TileContext(nc) as tc:
        _kernel_body(tc, out, xot, xs, wsr, wot, bvec)
    nc.compile()
    return nc


_PROGRAM = None


def _get_program():
    global _PROGRAM
    if _PROGRAM is None:
        _PROGRAM = build_program()
    return _PROGRAM


def make_in_maps(prop_state, W, b):
    prop = np.asarray(prop_state, dtype=np.float32).astype(np.float16)
    w = np.asarray(W, dtype=np.float32).reshape(2 * D)
    w_o, w_s = w[:D], w[D:]
    wsr = np.ascontiguousarray(
        np.broadcast_to(w_s.astype(np.float16)[None, :], (P, D)))
    wot = np.ascontiguousarray(w_o.reshape(NE, P).T.astype(np.float32))
    bv = np.ascontiguousarray(np.asarray(b, dtype=np.float32).reshape(1, 1))
    maps = []
    for i in range(NCORES):
        xot = np.ascontiguousarray(prop[i, :S].T)         # [512, 2048]
        # permute simp rows so HBM row p*NT+n = simp row n*P+p (contiguous
        # per-partition descriptor lines for the partition-inner layout)
        xs = np.ascontiguousarray(
            prop[i, S:].reshape(NT, P, D).transpose(1, 0, 2).reshape(S, D))
        maps.append({"xot": xot, "xs": xs, "wsr": wsr, "wot": wot,
                     "bvec": bv})
    return maps


def kernel(A, prop_state, W, b, _trace=False):
    nc = _get_program()
    in_maps = make_in_maps(prop_state, W, b)
    res = bass_utils.run_bass_kernel_spmd(
        nc, in_maps, core_ids=list(range(NCORES)), trace=_trace)
    out = np.stack([res.results[i]["out"] for i in range(NCORES)], axis=0)
    if _trace:
        kernel.last_results = res
    return out.astype(np.float32)


# revision 13
# speedup vs baseline: 1.5955x; 1.5955x over previous
"""Trainium2 Bass kernel for nn_AlignModel.

Computes out[b, j, i] = sigmoid(simp[b,j]·w_s + orig[b,i]·w_o + bias) where
orig/simp are the two halves of prop_state[b] ([B, 2S, D] -> [B,S,D] each),
w_o = W[0,:D], w_s = W[0,D:].

Sharding: data-parallel over batch B=8 across the 8 NeuronCores.  Host-side
staging per core (layout only -- all compute is on device):
  xot  [512, 2048] f16 = orig(b).T           (d-major, so PE can contract d)
  xs   [2048, 512] f16 = simp(b), rows permuted so HBM row p*16+n holds
        simp row n*128+p (partition-inner layout, contiguous descriptor lines)
  wsr  [128, 512] f16 = w_s replicated on all partitions
  wot  [128, 4]  f32 = w_o chunk-major (wot[k,e] = w_o[e*128+k])
  bvec [1, 2]   f32 = [0.5*b, -b]

Two-lane tile production (the baseline's single-ScalarE sigmoid chain was the
32us critical path; ScalarE's ACT rate of (N+352)/1.2GHz caps any one engine
at ~2us per [128,2048] tile, so two independent lanes run concurrently):
  - PE: psum_so[p,i] = s_o[i] via 16 fp16 matmuls, wrep stationary (built
    on-device from wot; the replication broadcasts s_o to all partitions).
  - DVE dots: s_sb_mat[p,t] = simp[t*128+p]·w_s per 4-tile group as one
    batched fp16 mul (2x rate) + two binary-fold adds (2x) + short f32
    reduce; b is folded via the bias ports downstream, not here.
  - Lane A (10 tiles): ScalarE tanh(0.5*s_o + 0.5*(ss+b)) -> f32 scratch
    (f32 kills the fp16 (1+T)/2 cancellation; scores reach -4.4 where
    sigma~0.012), then GpSimd affine 0.5*T+0.5 -> fp16 out tile.  tanh and
    exp share one ACT table set => zero table switches.
  - Lane B (6 tiles): out = 1/(1+exp(-ss-b)*exp(-s_o)).  ScalarE makes
    eo/es once; DVE then runs, per tile, a fused (eo*(-es_col))-1 tensor
    scalar, a magic-number reciprocal seed (K - bits(d) via one int16
    tensor_tensor on bitcast views, K=0x7798), and one fp16 Newton step
    y0*(2+dneg*y0) as one scalar_tensor_tensor.  Max recip rel err 3.3e-3
    (HW-probed); E <= exp(4.43) ~ 84 keeps fp16 safe.  (nc.vector.
    reciprocal measured 12.9us/tile -- an iterative macro; tensor_tensor_
    reduce and ALU divide/pow hang or fail to compile on TRN2.)
  - Stores: lane A rides the scalar HWDGE queue interleaved lag-1 with the
    tanh ACTIVATEs (each dma's GpSimd producer finished one ACT earlier =>
    no sequencer stalls); lane B rides the sync queue after the loads in
    DVE completion order.  xs load groups put lane-B's bias columns first
    so es (and the whole B chain) un-gates as early as possible.
"""

import numpy as np

import concourse.mybir as mybir
from concourse import bacc, bass_utils
from concourse.tile import TileContext

P = 128          # partitions
D = 512          # feature dim
S = 2048         # sents
NT = S // P      # 16 row-tiles
NE = D // P      # 4 contraction chunks
NS = 10          # lane-A (ScalarE tanh) tiles; lane B (DVE) gets NT-NS
RMAGIC = 0x7798  # fp16 reciprocal seed: bits(1/d) ~ RMAGIC - bits(d)
NCORES = 8
F32 = mybir.dt.float32
F16 = mybir.dt.float16
I16 = mybir.dt.int16
AF = mybir.ActivationFunctionType
ALU = mybir.AluOpType

# xs load groups: lane-B bias columns (10..15) land first so es un-gates
# early; lane-A columns 2..9 follow (consumed at ScalarE's 2us cadence).
XS_GROUPS = [(0, 2), (10, 16), (2, 6), (6, 10)]


def _kernel_body(tc, out, xot, xs, wsr, wot, bvec):
    nc = tc.nc
    xs_re = xs.rearrange("(p n) d -> p n d", n=NT)

    with (
        tc.tile_pool(name="consts", bufs=1) as cpool,
        tc.tile_pool(name="xin", bufs=1) as xpool,
        tc.tile_pool(name="prod", bufs=2) as prpool,
        tc.tile_pool(name="tpool", bufs=3) as tpool,
        tc.tile_pool(name="epool", bufs=2) as epool,
        tc.tile_pool(name="outbuf", bufs=1) as opool,
        tc.tile_pool(name="psum", bufs=1, space="PSUM") as ppool,
    ):
        # preload the exp_and_others ACT table set (covers Exp AND Tanh;
        # the whole kernel stays in this set -> no mid-kernel table loads).
        dummy = cpool.tile([1, 1], F32, tag="dummy")
        nc.vector.memset(dummy, 0.0)
        nc.scalar.activation(dummy, dummy, AF.Exp)

        # tiny const loads on the scalar HWDGE queue
        b2_sb = cpool.tile([P, 2], F32, tag="b2")     # [0.5b, -b]
        wsr_sb = cpool.tile([P, D], F16, tag="wsr")
        wot_sb = cpool.tile([P, NE], F32, tag="wot")
        nc.scalar.dma_start(out=wsr_sb, in_=wsr)
        nc.scalar.dma_start(out=wot_sb, in_=wot)
        nc.scalar.dma_start(out=b2_sb, in_=bvec.broadcast_to([P, 2]))
        b_half = b2_sb[:, 0:1]
        b_neg = b2_sb[:, 1:2]

        # build wrep on device: wrep[k, e*128+m] = w_o[e*128+k] (stationary
        # replicated along the PE output dim m so the matmul broadcasts s_o
        # to all partitions).  Replaces a 256KB wcat HBM load.
        ones = cpool.tile([P, P], F16, tag="ones")
        wrep_sb = cpool.tile([P, NE, P], F16, tag="wrep")
        nc.vector.memset(ones, 1.0)
        for e in range(NE):
            nc.vector.tensor_scalar_mul(wrep_sb[:, e, :], ones,
                                        wot_sb[:, e:e + 1])
        # K'' tile for the lane-B reciprocal seed: K + 0x8000 as int16
        # (dneg is negative, so K - bits(d) == K'' - bits(dneg) mod 2^16)
        ktile = cpool.tile([P, S], F16, tag="ktile")
        nc.vector.memset(ktile.bitcast(I16), RMAGIC - 0x8000)

        # --- input stream (sync queue, FIFO) ---
        xs_all = xpool.tile([P, NT, D], F16, tag="xs")
        xot_all = xpool.tile([P, NE, S], F16, tag="xot")
        nc.sync.dma_start(out=xs_all[:, 0:2, :], in_=xs_re[:, 0:2, :])
        for e in range(NE):
            nc.sync.dma_start(out=xot_all[:, e, :],
                              in_=xot[e * P:(e + 1) * P, :])
        for lo, hi in XS_GROUPS[1:]:
            nc.sync.dma_start(out=xs_all[:, lo:hi, :], in_=xs_re[:, lo:hi, :])

        s_sb_mat = cpool.tile([P, NT], F32, tag="ssmat")    # ss (no b)
        ss_half = cpool.tile([P, NT], F32, tag="sshalf")    # 0.5*ss + 0.5*b
        esn_mat = cpool.tile([P, NT - NS], F32, tag="esn")  # -exp(-(ss+b))
        eo_sb = cpool.tile([P, S], F16, tag="eo")           # exp(-s_o)
        so_psum = ppool.tile([P, S], F32, tag="so")

        # --- PE: s_o broadcast into PSUM ---
        for e in range(NE):
            for j in range(S // 512):
                nc.tensor.matmul(so_psum[:, j * 512:(j + 1) * 512],
                                 wrep_sb[:, e, :],
                                 xot_all[:, e, j * 512:(j + 1) * 512],
                                 start=(e == 0), stop=(e == NE - 1))

        # --- DVE dots: batched mul at 2x + two binary folds at 2x + short
        # f32 reduce; then one fused ts makes the lane-A half-bias column.
        for gi, (lo, hi) in enumerate(XS_GROUPS):
            g = hi - lo
            prod = prpool.tile([P, 6, D], F16, tag="prod", name=f"pr{gi}")
            nc.vector.tensor_mul(
                out=prod[:, 0:g, :],
                in0=xs_all[:, lo:hi, :],
                in1=wsr_sb.rearrange("p (a d) -> p a d", a=1).broadcast_to(
                    [P, g, D]))
            pr3 = prod.rearrange("p a (h q) -> p a h q", h=2)
            nc.vector.tensor_add(
                out=pr3[:, 0:g, 0, :],
                in0=pr3[:, 0:g, 0, :],
                in1=pr3[:, 0:g, 1, :])
            pr4 = prod.rearrange("p a (h q) -> p a h q", h=4)
            nc.vector.tensor_add(
                out=pr4[:, 0:g, 0, :],
                in0=pr4[:, 0:g, 0, :],
                in1=pr4[:, 0:g, 1, :])
            nc.vector.tensor_reduce(
                s_sb_mat[:, lo:hi], pr4[:, 0:g, 0, :],
                axis=mybir.AxisListType.X, op=ALU.add)
            nc.vector.tensor_scalar(
                out=ss_half[:, lo:hi], in0=s_sb_mat[:, lo:hi],
                scalar1=0.5, scalar2=b_half, op0=ALU.mult, op1=ALU.add)

        out_all = opool.tile([P, NT, S], F16, tag="oall")

        # --- ScalarE program (issue order == execution order), all in the
        # exp_and_others table set:
        #   eo, T0, es(B cols), T1..T9 with lag-1 A-store interleave
        def lane_a_tile(t):
            T = tpool.tile([P, S], F32, tag="T", name=f"T{t}")
            nc.scalar.activation(T, so_psum, AF.Tanh,
                                 bias=ss_half[:, t:t + 1], scale=0.5)
            nc.gpsimd.tensor_scalar(out=out_all[:, t, :], in0=T,
                                    scalar1=0.5, scalar2=0.5,
                                    op0=ALU.mult, op1=ALU.add)

        def store_a(t):
            nc.scalar.dma_start(out=out[t * P:(t + 1) * P, :],
                                in_=out_all[:, t, :])

        nc.scalar.activation(eo_sb, so_psum, AF.Exp, scale=-1.0)
        lane_a_tile(0)
        # es for lane-B columns: exp(-(ss+b)); negated below on DVE so the
        # seed/Newton signs work out with one op less per tile.
        nc.scalar.activation(esn_mat, s_sb_mat[:, NS:NT], AF.Exp,
                             bias=b_neg, scale=-1.0)
        lane_a_tile(1)
        lane_a_tile(2)
        store_a(0)
        for t in range(3, NS):
            lane_a_tile(t)
            store_a(t - 2)
        store_a(NS - 2)
        store_a(NS - 1)

        # --- DVE lane B ---
        with nc.allow_low_precision(reason="sigmoid output tile in fp16"):
            nc.vector.tensor_scalar_mul(esn_mat, esn_mat, -1.0)
            for t in range(NS, NT):
                k = t - NS
                dneg = epool.tile([P, S], F16, tag="E", name=f"E{t}")
                y0 = epool.tile([P, S], F16, tag="y0", name=f"y{t}")
                # dneg = -(1 + eo*es) = eo*(-es) - 1
                nc.vector.tensor_scalar(out=dneg, in0=eo_sb,
                                        scalar1=esn_mat[:, k:k + 1],
                                        scalar2=-1.0,
                                        op0=ALU.mult, op1=ALU.add)
                # y0 bits = K'' - bits(dneg)  (magic reciprocal seed)
                nc.vector.tensor_tensor(out=y0.bitcast(I16),
                                        in0=ktile.bitcast(I16),
                                        in1=dneg.bitcast(I16),
                                        op=ALU.subtract)
                # one Newton step: y1 = y0*(2 - d*y0) = (dneg*y0 + 2)*y0
                nc.vector.tensor_tensor(out=dneg, in0=dneg, in1=y0,
                                        op=ALU.mult)
                nc.vector.scalar_tensor_tensor(
                    out=out_all[:, t, :], in0=dneg, scalar=2.0, in1=y0,
                    op0=ALU.add, op1=ALU.mult)
                if k % 2 == 1:
                    r0 = (t - 1) * P
                    dst = out[r0:r0 + 2 * P, :].rearrange(
                        "(q p) i -> p q i", p=P)
                    nc.sync.dma_start(out=dst,
                                      in_=out_all[:, t - 1:t + 1, :])


def build_program():
    nc = bacc.Bacc(
        "TRN2",
        debug=False,
        target_bir_lowering=False,
        num_devices=NCORES,
    )
    xot = nc.dram_tensor("xot", [D, S], F16, kind="ExternalInput").ap()
    xs = nc.dram_tensor("xs", [S, D], F16, kind="ExternalInput").ap()
    wsr = nc.dram_tensor("wsr", [P, D], F16, kind="ExternalInput").ap()
    wot = nc.dram_tensor("wot", [P, NE], F32, kind="ExternalInput").ap()
    bvec = nc.dram_tensor("bvec", [1, 2], F32, kind="ExternalInput").ap()
    out = nc.dram_tensor("out", [S, S], F16, kind="ExternalOutput").ap()
    with TileContext(nc) as tc:
        _kernel_body(tc, out, xot, xs, wsr, wot, bvec)
    nc.compile()
    return nc


_PROGRAM = None


def _get_program():
    global _PROGRAM
    if _PROGRAM is None:
        _PROGRAM = build_program()
    return _PROGRAM


def make_in_maps(prop_state, W, b):
    prop = np.asarray(prop_state, dtype=np.float32).astype(np.float16)
    w = np.asarray(W, dtype=np.float32).reshape(2 * D)
    w_o, w_s = w[:D], w[D:]
    wsr = np.ascontiguousarray(
        np.broadcast_to(w_s.astype(np.float16)[None, :], (P, D)))
    wot = np.ascontiguousarray(w_o.reshape(NE, P).T.astype(np.float32))
    bval = float(np.asarray(b, dtype=np.float32).reshape(-1)[0])
    bv = np.ascontiguousarray(
        np.array([[0.5 * bval, -bval]], dtype=np.float32))
    maps = []
    for i in range(NCORES):
        xot = np.ascontiguousarray(prop[i, :S].T)         # [512, 2048]
        # permute simp rows so HBM row p*NT+n = simp row n*P+p (contiguous
        # per-partition descriptor lines for the partition-inner layout)
        xs = np.ascontiguousarray(
            prop[i, S:].reshape(NT, P, D).transpose(1, 0, 2).reshape(S, D))
        maps.append({"xot": xot, "xs": xs, "wsr": wsr, "wot": wot,
                     "bvec": bv})
    return maps


def kernel(A, prop_state, W, b, _trace=False):
    nc = _get_program()
    in_maps = make_in_maps(prop_state, W, b)
    res = bass_utils.run_bass_kernel_spmd(
        nc, in_maps, core_ids=list(range(NCORES)), trace=_trace)
    out = np.stack([res.results[i]["out"] for i in range(NCORES)], axis=0)
    if _trace:
        kernel.last_results = res
    return out.astype(np.float32)


# revision 14
# speedup vs baseline: 2.1060x; 1.3200x over previous
"""Trainium2 Bass kernel for nn_AlignModel.

Computes out[b, j, i] = sigmoid(simp[b,j]·w_s + orig[b,i]·w_o + bias) where
orig/simp are the two halves of prop_state[b] ([B, 2S, D] -> [B,S,D] each),
w_o = W[0,:D], w_s = W[0,D:].

Sharding: data-parallel over batch B=8 across the 8 NeuronCores.  Host-side
staging per core (layout only -- all compute is on device):
  xot  [512, 2048] f16 = orig(b).T           (d-major, so PE can contract d)
  xs   [2048, 512] f16 = simp(b), rows permuted so HBM row p*16+n holds
        simp row n*128+p (partition-inner layout, contiguous descriptor lines)
  wsr  [128, 512] f16 = w_s replicated on all partitions
  wot  [128, 4]  f32 = w_o chunk-major (wot[k,e] = w_o[e*128+k])
  bvec [1, 1]   f32

Architecture notes (HW-measured on this part):
  - ScalarE ACTIVATE runs (N+352)/1.2GHz regardless of dtype: 2.0us per
    [128,2048] tile, 32us for all 16 -- the hard production wall.  Every
    alternative producer measured worse: DVE reciprocal() is a 12.9us/tile
    macro; DVE 2-input ops run ~1.2us/pass (no 2x), so any Newton/exp
    decomposition needs >=4.6us/tile; GpSimd tensor_scalar shares an SBUF
    port with DVE and the two slow each other ~2x when concurrent.
    So: one lane, ScalarE sigmoids, and optimize everything around it.
  - vs the previous kernel: sigmoids start at ~16us instead of ~20 (no
    256KB wcat load ahead of xot -- wrep is built on-device from a 2KB
    wot; xot rides first on the sync queue in 0.25MiB chunks so the PE
    chain starts and finishes earlier), and each tile's store issues
    zero-lag on the scalar HWDGE queue right after its own ACTIVATE
    (producer==issuer, FIFO never stalls; per-tile 0.5MiB stores kill the
    4us starvation gaps the grouped-store schedule had).
  - PE: psum_so[p,i] = s_o[i] via 16 K=128/N=512 fp16 matmuls (wrep
    stationary, replicated along the output dim so the matmul broadcasts
    s_o to all partitions).  b is folded into the bias columns.
  - DVE dots: s_sb[p,t] = simp[t*128+p]·w_s + b per 4..6-tile group: one
    batched fp16 mul, two binary-fold adds (halve the reduce length), a
    short reduce, and a fused (x1,+b) tensor_scalar -> bias columns stay
    comfortably ahead of ScalarE's 2us cadence.
"""

import numpy as np

import concourse.mybir as mybir
from concourse import bacc, bass_utils
from concourse.tile import TileContext

P = 128          # partitions
D = 512          # feature dim
S = 2048         # sents
NT = S // P      # 16 row-tiles
NE = D // P      # 4 contraction chunks
NCORES = 8
F32 = mybir.dt.float32
F16 = mybir.dt.float16
AF = mybir.ActivationFunctionType
ALU = mybir.AluOpType

XS_GROUPS = [(0, 2), (2, 6), (6, 10), (10, NT)]


def _kernel_body(tc, out, xot, xs, wsr, wot, bvec):
    nc = tc.nc
    xs_re = xs.rearrange("(p n) d -> p n d", n=NT)

    with (
        tc.tile_pool(name="consts", bufs=1) as cpool,
        tc.tile_pool(name="xin", bufs=1) as xpool,
        tc.tile_pool(name="prod", bufs=2) as prpool,
        tc.tile_pool(name="outbuf", bufs=1) as opool,
        tc.tile_pool(name="psum", bufs=1, space="PSUM") as ppool,
    ):
        # preload the sigmoid ACT table set via a dep-free dummy at t~0
        dummy = cpool.tile([1, 1], F32, tag="dummy")
        nc.vector.memset(dummy, 0.0)
        nc.scalar.activation(dummy, dummy, AF.Sigmoid)

        # tiny const loads on the scalar HWDGE queue (land ~9us, before
        # the first dot group / first matmul needs them)
        b_sb = cpool.tile([P, 1], F32, tag="bsb")
        wsr_sb = cpool.tile([P, D], F16, tag="wsr")
        wot_sb = cpool.tile([P, NE], F32, tag="wot")
        nc.scalar.dma_start(out=wsr_sb, in_=wsr)
        nc.scalar.dma_start(out=wot_sb, in_=wot)
        nc.scalar.dma_start(out=b_sb, in_=bvec.broadcast_to([P, 1]))

        # build wrep on device: wrep[k, e*128+m] = w_o[e*128+k]
        ones = cpool.tile([P, P], F16, tag="ones")
        wrep_sb = cpool.tile([P, NE, P], F16, tag="wrep")
        nc.vector.memset(ones, 1.0)
        for e in range(NE):
            nc.vector.tensor_scalar_mul(wrep_sb[:, e, :], ones,
                                        wot_sb[:, e:e + 1])

        # --- input stream (sync queue, FIFO): 2 simp tiles (un-gates the
        # first dot group), xot in 0.25MiB half-chunks (earlier PE start
        # AND earlier per-chunk completion sems), then simp rest ---
        xs_all = xpool.tile([P, NT, D], F16, tag="xs")
        xot_all = xpool.tile([P, NE, S], F16, tag="xot")
        nc.sync.dma_start(out=xs_all[:, 0:2, :], in_=xs_re[:, 0:2, :])
        for e in range(NE):
            for h in range(2):
                nc.sync.dma_start(
                    out=xot_all[:, e, h * 1024:(h + 1) * 1024],
                    in_=xot[e * P:(e + 1) * P, h * 1024:(h + 1) * 1024])
        for lo, hi in XS_GROUPS[1:]:
            nc.sync.dma_start(out=xs_all[:, lo:hi, :], in_=xs_re[:, lo:hi, :])

        s_sb_mat = cpool.tile([P, NT], F32, tag="ssmat")   # ss (raw)
        ssb_mat = cpool.tile([P, NT], F32, tag="ssb")      # ss + b
        so_psum = ppool.tile([P, S], F32, tag="so")

        # --- PE: s_o broadcast into PSUM (half-chunk strips so each
        # matmul's input sem arrives as early as possible) ---
        for e in range(NE):
            for j in range(S // 512):
                nc.tensor.matmul(so_psum[:, j * 512:(j + 1) * 512],
                                 wrep_sb[:, e, :],
                                 xot_all[:, e, j * 512:(j + 1) * 512],
                                 start=(e == 0), stop=(e == NE - 1))

        # --- DVE dots: batched mul + two binary folds + short reduce ---
        for gi, (lo, hi) in enumerate(XS_GROUPS):
            g = hi - lo
            prod = prpool.tile([P, 6, D], F16, tag="prod", name=f"pr{gi}")
            nc.vector.tensor_mul(
                out=prod[:, 0:g, :],
                in0=xs_all[:, lo:hi, :],
                in1=wsr_sb.rearrange("p (a d) -> p a d", a=1).broadcast_to(
                    [P, g, D]))
            pr3 = prod.rearrange("p a (h q) -> p a h q", h=2)
            nc.vector.tensor_add(
                out=pr3[:, 0:g, 0, :], in0=pr3[:, 0:g, 0, :],
                in1=pr3[:, 0:g, 1, :])
            pr4 = prod.rearrange("p a (h q) -> p a h q", h=4)
            nc.vector.tensor_add(
                out=pr4[:, 0:g, 0, :], in0=pr4[:, 0:g, 0, :],
                in1=pr4[:, 0:g, 1, :])
            nc.vector.tensor_reduce(
                s_sb_mat[:, lo:hi], pr4[:, 0:g, 0, :],
                axis=mybir.AxisListType.X, op=ALU.add)
            nc.vector.tensor_scalar(
                out=ssb_mat[:, lo:hi], in0=s_sb_mat[:, lo:hi],
                scalar1=1.0, scalar2=b_sb, op0=ALU.mult, op1=ALU.add)

        out_all = opool.tile([P, NT, S], F16, tag="oall")

        # --- ScalarE: 16 sigmoids, each followed zero-lag by its own
        # store on the same (scalar) HWDGE queue ---
        for t in range(NT):
            nc.scalar.activation(out_all[:, t, :], so_psum, AF.Sigmoid,
                                 bias=ssb_mat[:, t:t + 1], scale=1.0)
            nc.scalar.dma_start(out=out[t * P:(t + 1) * P, :],
                                in_=out_all[:, t, :])


def build_program():
    nc = bacc.Bacc(
        "TRN2",
        debug=False,
        target_bir_lowering=False,
        num_devices=NCORES,
    )
    xot = nc.dram_tensor("xot", [D, S], F16, kind="ExternalInput").ap()
    xs = nc.dram_tensor("xs", [S, D], F16, kind="ExternalInput").ap()
    wsr = nc.dram_tensor("wsr", [P, D], F16, kind="ExternalInput").ap()
    wot = nc.dram_tensor("wot", [P, NE], F32, kind="ExternalInput").ap()
    bvec = nc.dram_tensor("bvec", [1, 1], F32, kind="ExternalInput").ap()
    out = nc.dram_tensor("out", [S, S], F16, kind="ExternalOutput").ap()
    with TileContext(nc) as tc:
        _kernel_body(tc, out, xot, xs, wsr, wot, bvec)
    nc.compile()
    return nc


_PROGRAM = None


def _get_program():
    global _PROGRAM
    if _PROGRAM is None:
        _PROGRAM = build_program()
    return _PROGRAM


def make_in_maps(prop_state, W, b):
    prop = np.asarray(prop_state, dtype=np.float32).astype(np.float16)
    w = np.asarray(W, dtype=np.float32).reshape(2 * D)
    w_o, w_s = w[:D], w[D:]
    wsr = np.ascontiguousarray(
        np.broadcast_to(w_s.astype(np.float16)[None, :], (P, D)))
    wot = np.ascontiguousarray(w_o.reshape(NE, P).T.astype(np.float32))
    bv = np.ascontiguousarray(np.asarray(b, dtype=np.float32).reshape(1, 1))
    maps = []
    for i in range(NCORES):
        xot = np.ascontiguousarray(prop[i, :S].T)         # [512, 2048]
        xs = np.ascontiguousarray(
            prop[i, S:].reshape(NT, P, D).transpose(1, 0, 2).reshape(S, D))
        maps.append({"xot": xot, "xs": xs, "wsr": wsr, "wot": wot,
                     "bvec": bv})
    return maps


def kernel(A, prop_state, W, b, _trace=False):
    nc = _get_program()
    in_maps = make_in_maps(prop_state, W, b)
    res = bass_utils.run_bass_kernel_spmd(
        nc, in_maps, core_ids=list(range(NCORES)), trace=_trace)
    out = np.stack([res.results[i]["out"] for i in range(NCORES)], axis=0)
    if _trace:
        kernel.last_results = res
    return out.astype(np.float32)


# revision 16
# speedup vs baseline: 2.3230x; 1.1030x over previous
"""Trainium2 Bass kernel for nn_AlignModel.

Computes out[b, j, i] = sigmoid(simp[b,j]·w_s + orig[b,i]·w_o + bias) where
orig/simp are the two halves of prop_state[b] ([B, 2S, D] -> [B,S,D] each),
w_o = W[0,:D], w_s = W[0,D:].

Sharding: data-parallel over batch B=8 across the 8 NeuronCores.  Host-side
staging per core (layout only -- all compute is on device):
  xot  [512, 2048] f16 = orig(b).T           (d-major, so PE can contract d)
  xs   [2048, 512] f16 = simp(b), rows permuted so HBM row p*16+n holds
        simp row n*128+p (partition-inner layout, contiguous descriptor lines)
  wsr  [128, 512] f16 = w_s replicated on all partitions
  wot  [128, 4]  f32 = w_o chunk-major (wot[k,e] = w_o[e*128+k])
  bvec [1, 1]   f32

Architecture notes (HW-measured on this part):
  - ScalarE ACTIVATE runs (N+352)/1.2GHz regardless of dtype: 2.0us per
    [128,2048] tile, 32us for all 16 -- the hard production wall.  Every
    alternative producer measured worse: DVE reciprocal() is a 12.9us/tile
    macro; DVE 2-input ops run ~1.2us/pass (no 2x), so any Newton/exp
    decomposition needs >=4.6us/tile; GpSimd tensor_scalar shares an SBUF
    port with DVE and the two slow each other ~2x when concurrent.
    So: one lane, ScalarE sigmoids, and optimize everything around it.
  - vs the previous kernel: sigmoids start at ~16us instead of ~20 (no
    256KB wcat load ahead of xot -- wrep is built on-device from a 2KB
    wot; xot rides first on the sync queue in 0.25MiB chunks so the PE
    chain starts and finishes earlier), and each tile's store issues
    zero-lag on the scalar HWDGE queue right after its own ACTIVATE
    (producer==issuer, FIFO never stalls; per-tile 0.5MiB stores kill the
    4us starvation gaps the grouped-store schedule had).
  - PE: psum_so[p,i] = s_o[i] via 16 K=128/N=512 fp16 matmuls (wrep
    stationary, replicated along the output dim so the matmul broadcasts
    s_o to all partitions).  b is folded into the bias columns.
  - DVE dots: s_sb[p,t] = simp[t*128+p]·w_s + b per 4..6-tile group: one
    batched fp16 mul, two binary-fold adds (halve the reduce length), a
    short reduce, and a fused (x1,+b) tensor_scalar -> bias columns stay
    comfortably ahead of ScalarE's 2us cadence.
"""

import numpy as np

import concourse.mybir as mybir
from concourse import bacc, bass_utils
from concourse.tile import TileContext

P = 128          # partitions
D = 512          # feature dim
S = 2048         # sents
NT = S // P      # 16 row-tiles
NE = D // P      # 4 contraction chunks
NCORES = 8
F32 = mybir.dt.float32
F16 = mybir.dt.float16
AF = mybir.ActivationFunctionType
ALU = mybir.AluOpType

XS_GROUPS = [(0, 2), (2, 6), (6, 10), (10, NT)]


def _kernel_body(tc, out, xot, xs, wsr, wot, bvec):
    nc = tc.nc
    xs_re = xs.rearrange("(p n) d -> p n d", n=NT)

    with (
        tc.tile_pool(name="consts", bufs=1) as cpool,
        tc.tile_pool(name="xin", bufs=1) as xpool,
        tc.tile_pool(name="prod", bufs=2) as prpool,
        tc.tile_pool(name="outbuf", bufs=1) as opool,
        tc.tile_pool(name="psum", bufs=1, space="PSUM") as ppool,
    ):
        # preload the sigmoid ACT table set via a dep-free dummy at t~0
        dummy = cpool.tile([1, 1], F32, tag="dummy")
        nc.vector.memset(dummy, 0.0)
        nc.scalar.activation(dummy, dummy, AF.Sigmoid)

        # tiny const loads on the scalar HWDGE queue (land ~9us, before
        # the first dot group / first matmul needs them)
        b_sb = cpool.tile([P, 1], F32, tag="bsb")
        wsr_sb = cpool.tile([P, D], F16, tag="wsr")
        wot_sb = cpool.tile([P, NE], F32, tag="wot")
        nc.scalar.dma_start(out=wsr_sb, in_=wsr)
        nc.scalar.dma_start(out=wot_sb, in_=wot)
        nc.scalar.dma_start(out=b_sb, in_=bvec.broadcast_to([P, 1]))

        # build wrep on device: wrep[k, e*128+m] = w_o[e*128+k]
        ones = cpool.tile([P, P], F16, tag="ones")
        wrep_sb = cpool.tile([P, NE, P], F16, tag="wrep")
        nc.vector.memset(ones, 1.0)
        for e in range(NE):
            nc.vector.tensor_scalar_mul(wrep_sb[:, e, :], ones,
                                        wot_sb[:, e:e + 1])

        # --- input stream: xot alone at the head of the sync queue (its
        # chunks gate the PE chain -> first sigmoid); the first 2 simp
        # tiles ride the (otherwise idle) scalar queue; simp rest follows
        # xot on sync.  Every DMA trigger costs ~0.6us of its sequencer,
        # so chunks stay at 0.5MiB granularity. ---
        xs_all = xpool.tile([P, NT, D], F16, tag="xs")
        xot_all = xpool.tile([P, NE, S], F16, tag="xot")
        nc.scalar.dma_start(out=xs_all[:, 0:2, :], in_=xs_re[:, 0:2, :])
        for e in range(NE):
            nc.sync.dma_start(out=xot_all[:, e, :],
                              in_=xot[e * P:(e + 1) * P, :])
        for lo, hi in XS_GROUPS[1:]:
            nc.sync.dma_start(out=xs_all[:, lo:hi, :], in_=xs_re[:, lo:hi, :])

        s_sb_mat = cpool.tile([P, NT], F32, tag="ssmat")   # ss (raw)
        ssb_mat = cpool.tile([P, NT], F32, tag="ssb")      # ss + b
        so_psum = ppool.tile([P, S], F32, tag="so")

        # --- PE: s_o broadcast into PSUM (half-chunk strips so each
        # matmul's input sem arrives as early as possible) ---
        for e in range(NE):
            for j in range(S // 512):
                nc.tensor.matmul(so_psum[:, j * 512:(j + 1) * 512],
                                 wrep_sb[:, e, :],
                                 xot_all[:, e, j * 512:(j + 1) * 512],
                                 start=(e == 0), stop=(e == NE - 1))

        # --- DVE dots: batched mul + two binary folds + short reduce ---
        for gi, (lo, hi) in enumerate(XS_GROUPS):
            g = hi - lo
            prod = prpool.tile([P, 6, D], F16, tag="prod", name=f"pr{gi}")
            nc.vector.tensor_mul(
                out=prod[:, 0:g, :],
                in0=xs_all[:, lo:hi, :],
                in1=wsr_sb.rearrange("p (a d) -> p a d", a=1).broadcast_to(
                    [P, g, D]))
            pr3 = prod.rearrange("p a (h q) -> p a h q", h=2)
            nc.vector.tensor_add(
                out=pr3[:, 0:g, 0, :], in0=pr3[:, 0:g, 0, :],
                in1=pr3[:, 0:g, 1, :])
            pr4 = prod.rearrange("p a (h q) -> p a h q", h=4)
            nc.vector.tensor_add(
                out=pr4[:, 0:g, 0, :], in0=pr4[:, 0:g, 0, :],
                in1=pr4[:, 0:g, 1, :])
            nc.vector.tensor_reduce(
                s_sb_mat[:, lo:hi], pr4[:, 0:g, 0, :],
                axis=mybir.AxisListType.X, op=ALU.add)
            nc.vector.tensor_scalar(
                out=ssb_mat[:, lo:hi], in0=s_sb_mat[:, lo:hi],
                scalar1=1.0, scalar2=b_sb, op0=ALU.mult, op1=ALU.add)

        out_all = opool.tile([P, NT, S], F16, tag="oall")

        # --- ScalarE: 16 back-to-back sigmoids (a dma_start here would eat
        # ~0.6us of ACT sequencer each); the per-tile stores ride the sync
        # queue, whose sequencer is idle once the loads have issued, in
        # ACT completion order (FIFO never stalls). ---
        for t in range(NT):
            nc.scalar.activation(out_all[:, t, :], so_psum, AF.Sigmoid,
                                 bias=ssb_mat[:, t:t + 1], scale=1.0)
            nc.sync.dma_start(out=out[t * P:(t + 1) * P, :],
                              in_=out_all[:, t, :])


def build_program():
    nc = bacc.Bacc(
        "TRN2",
        debug=False,
        target_bir_lowering=False,
        num_devices=NCORES,
    )
    xot = nc.dram_tensor("xot", [D, S], F16, kind="ExternalInput").ap()
    xs = nc.dram_tensor("xs", [S, D], F16, kind="ExternalInput").ap()
    wsr = nc.dram_tensor("wsr", [P, D], F16, kind="ExternalInput").ap()
    wot = nc.dram_tensor("wot", [P, NE], F32, kind="ExternalInput").ap()
    bvec = nc.dram_tensor("bvec", [1, 1], F32, kind="ExternalInput").ap()
    out = nc.dram_tensor("out", [S, S], F16, kind="ExternalOutput").ap()
    with TileContext(nc) as tc:
        _kernel_body(tc, out, xot, xs, wsr, wot, bvec)
    nc.compile()
    return nc


_PROGRAM = None


def _get_program():
    global _PROGRAM
    if _PROGRAM is None:
        _PROGRAM = build_program()
    return _PROGRAM


def make_in_maps(prop_state, W, b):
    prop = np.asarray(prop_state, dtype=np.float32).astype(np.float16)
    w = np.asarray(W, dtype=np.float32).reshape(2 * D)
    w_o, w_s = w[:D], w[D:]
    wsr = np.ascontiguousarray(
        np.broadcast_to(w_s.astype(np.float16)[None, :], (P, D)))
    wot = np.ascontiguousarray(w_o.reshape(NE, P).T.astype(np.float32))
    bv = np.ascontiguousarray(np.asarray(b, dtype=np.float32).reshape(1, 1))
    maps = []
    for i in range(NCORES):
        xot = np.ascontiguousarray(prop[i, :S].T)         # [512, 2048]
        xs = np.ascontiguousarray(
            prop[i, S:].reshape(NT, P, D).transpose(1, 0, 2).reshape(S, D))
        maps.append({"xot": xot, "xs": xs, "wsr": wsr, "wot": wot,
                     "bvec": bv})
    return maps


def kernel(A, prop_state, W, b, _trace=False):
    nc = _get_program()
    in_maps = make_in_maps(prop_state, W, b)
    res = bass_utils.run_bass_kernel_spmd(
        nc, in_maps, core_ids=list(range(NCORES)), trace=_trace)
    out = np.stack([res.results[i]["out"] for i in range(NCORES)], axis=0)
    if _trace:
        kernel.last_results = res
    return out.astype(np.float32)
